# revision 1
# baseline (speedup 1.0000x reference)
"""Trainium2 Bass kernel for nn_BGraphConvolution (GCN message passing).

Computes out = segment_sum(vals * x[cols]) @ bjorck_orthonormalize(weight),
using the associativity out = (A_sp @ x) @ W_ortho:
  - 8-way shard over destination nodes (adj_rows is sorted, so each core
    owns a contiguous edge slab).
  - Per 128-dest window: dma_gather source rows of x from HBM (int16
    indices -> 4 source chunks of 32768 rows), build a one-hot
    stationary matrix M[e, d] = val[e] * (off[e] == d) on the DVE, and
    accumulate PSUM[d, f] += M.T @ G on the PE.
  - Transpose the window accumulator with PE transposes, then multiply
    by the (host-precomputed, fp64 Bjorck) orthonormalized weight.
The Bjorck orthonormalization is a cheap 512x512 fixed-point iteration;
it is computed once on the host in fp64 (exact) and shipped to every
core, per the "replicate the small weight orthonormalization" sharding
strategy.
"""

import os
import sys

for _p in ("/opt/trn_rl_repo", os.path.expanduser("~/.axon_site/_ro/trn_rl_repo")):
    if os.path.isdir(_p) and _p not in sys.path:
        sys.path.insert(0, _p)

import numpy as np

import concourse.mybir as mybir
import concourse.tile as tile
from concourse import bacc
from concourse.bass_utils import run_bass_kernel_spmd

P = 128
N_NODES = 100000
N_FEAT = 512
N_CORES = 8
CHUNK = 32768  # int16 gather index range
BJORCK_BETA = 0.5
BJORCK_ITERS = 10
MAX_GATHER_BLOCKS = 8  # 1024 idxs per dma_gather (SWDGE ring limit)

F32 = mybir.dt.float32
F32R = mybir.dt.float32r
I16 = mybir.dt.int16


def _bjorck_host(weight: np.ndarray) -> np.ndarray:
    """ortho_w in fp64 exactly as the reference defines it."""
    s = float(np.float32(np.sqrt(weight.shape[0] * weight.shape[1])))
    w = weight.astype(np.float64).T / s
    for _ in range(BJORCK_ITERS):
        w = (1.0 + BJORCK_BETA) * w - BJORCK_BETA * (w @ (w.T @ w))
    return np.ascontiguousarray(w.T.astype(np.float32))


def _prep_host(adj_rows, adj_cols, adj_vals, n_nodes, n_cores):
    """Shard edges by destination, group per (dest-window, src-chunk),
    pad each group to 128-blocks with a SHARED (cross-core) structure.

    Returns (meta, per_core) where meta describes the shared program
    structure and per_core holds each core's padded streams.
    """
    rows_per_core = n_nodes // n_cores
    n_win = (rows_per_core + P - 1) // P
    n_chunk = (n_nodes + CHUNK - 1) // CHUNK

    cores = []
    counts = np.zeros((n_cores, n_win * n_chunk), dtype=np.int64)
    for c in range(n_cores):
        lo, hi = np.searchsorted(adj_rows, [c * rows_per_core, (c + 1) * rows_per_core])
        r = adj_rows[lo:hi].astype(np.int64) - c * rows_per_core
        col = adj_cols[lo:hi].astype(np.int64)
        v = adj_vals[lo:hi]
        w = r // P
        ch = col // CHUNK
        key = w * n_chunk + ch
        order = np.argsort(key, kind="stable")
        cores.append((r[order], col[order], v[order], key[order]))
        counts[c] = np.bincount(key, minlength=n_win * n_chunk)

    gmax = counts.max(axis=0)  # shared max per (window, chunk)
    nblk = (gmax + P - 1) // P  # blocks per group (0 = skip group)
    tot_blk = int(nblk.sum())

    # shared structure: per window -> list of (chunk, blk0, nblocks)
    windows = []
    blk0 = 0
    gathers = []  # (key, slice_start_block_global, gb) in program order
    for w in range(n_win):
        groups = []
        for ch in range(n_chunk):
            nb = int(nblk[w * n_chunk + ch])
            if nb:
                groups.append((ch, blk0, nb))
                for i in range(0, nb, MAX_GATHER_BLOCKS):
                    gb = min(MAX_GATHER_BLOCKS, nb - i)
                    gathers.append((w * n_chunk + ch, blk0 + i, gb))
                blk0 += nb
        windows.append(groups)
    assert blk0 == tot_blk

    # per-core padded streams
    group_start = np.zeros(n_win * n_chunk + 1, dtype=np.int64)
    np.cumsum(nblk * P, out=group_start[1:])
    tot = tot_blk * P
    per_core = []
    for c in range(n_cores):
        r, col, v, key = cores[c]
        gidx = np.full(tot, -1, dtype=np.int16)
        offs = np.zeros(tot, dtype=np.float32)
        vals = np.zeros(tot, dtype=np.float32)
        cnt = np.bincount(key, minlength=n_win * n_chunk)
        # position of each edge inside its (padded) group
        pos_in_group = np.arange(len(key)) - np.repeat(
            np.concatenate(([0], np.cumsum(cnt)))[:-1], cnt
        )
        dst = group_start[key] + pos_in_group
        gidx[dst] = (col % CHUNK).astype(np.int16)
        offs[dst] = (r % P).astype(np.float32)
        vals[dst] = v
        # per-gather valid counts (pads are -1 at each group tail and
        # are skipped by the DGE); empty slices get one forced idx 0
        nvalid = np.zeros(len(gathers), dtype=np.int32)
        for gi, (kk, sb, gb) in enumerate(gathers):
            local0 = (sb - (group_start[kk] // P)) * P
            valid = int(min(max(cnt[kk] - local0, 0), gb * P))
            if valid == 0:
                gidx[sb * P] = 0
                valid = 1
            nvalid[gi] = valid
        per_core.append(
            {
                "gidx": np.ascontiguousarray(
                    np.tile(gidx.reshape(-1, 16).T, (8, 1))
                ),  # [128, tot/16]
                "offs": np.ascontiguousarray(offs.reshape(-1, P).T),  # [128, tot_blk]
                "vals": np.ascontiguousarray(vals.reshape(-1, P).T),
                "nvalid": nvalid.reshape(1, -1).copy(),
            }
        )

    meta = {
        "gathers": gathers,
        "n_win": n_win,
        "n_chunk": n_chunk,
        "windows": windows,
        "tot_blk": tot_blk,
        "rows_per_core": rows_per_core,
        "n_nodes": n_nodes,
    }
    return meta, per_core


def _build_program(meta, precision="f32r", reps=1,
                   skip_compute=False, skip_gather=False):
    """Build the per-core Bass program (shared across all cores)."""
    n_win = meta["n_win"]
    windows = meta["windows"]
    tot_blk = meta["tot_blk"]
    rows_per_core = meta["rows_per_core"]
    n_nodes = meta["n_nodes"]

    gdt = {"f32r": F32R, "f32": F32, "f16": mybir.dt.float16}[precision]
    max_bw = max(sum(nb for _, _, nb in groups) for groups in windows)

    nc = bacc.Bacc(
        "TRN2",
        target_bir_lowering=False,
        debug=False,
        num_devices=1,
    )
    x_d = nc.dram_tensor("x", [n_nodes, N_FEAT], gdt, kind="ExternalInput").ap()
    wq_d = nc.dram_tensor("wq", [N_FEAT, N_FEAT], F32, kind="ExternalInput").ap()
    gidx_d = nc.dram_tensor("gidx", [P, tot_blk * 8], I16, kind="ExternalInput").ap()
    offs_d = nc.dram_tensor("offs", [P, tot_blk], F32, kind="ExternalInput").ap()
    vals_d = nc.dram_tensor("vals", [P, tot_blk], F32, kind="ExternalInput").ap()
    iota_d = nc.dram_tensor("iota", [P, P], F32, kind="ExternalInput").ap()
    ident_d = nc.dram_tensor("ident", [P, P], F32, kind="ExternalInput").ap()
    n_gathers = len(meta["gathers"])
    nv_d = nc.dram_tensor(
        "nvalid", [1, n_gathers], mybir.dt.int32, kind="ExternalInput"
    ).ap()
    out_d = nc.dram_tensor(
        "out", [rows_per_core, N_FEAT], F32, kind="ExternalOutput"
    ).ap()

    with tile.TileContext(nc) as tc:
        with (
            tc.tile_pool(name="const", bufs=1) as cpool,
            tc.tile_pool(name="gpool", bufs=3) as gpool,
            tc.tile_pool(name="mpool", bufs=6) as mpool,
            tc.tile_pool(name="wio", bufs=2) as wio,
            tc.tile_pool(name="evict", bufs=2) as ev,
            tc.tile_pool(name="psacc", bufs=2, space="PSUM") as psacc,
            tc.tile_pool(name="pstr", bufs=2, space="PSUM") as pstr,
            tc.tile_pool(name="psout", bufs=2, space="PSUM") as psout,
        ):
            iota_t = cpool.tile([P, P], F32)
            nc.sync.dma_start(iota_t[:], iota_d[:])
            ident_t = cpool.tile([P, P], F32)
            nc.sync.dma_start(ident_t[:], ident_d[:])
            wq_t = cpool.tile([P, 4, N_FEAT], F32)
            for kt in range(4):
                nc.sync.dma_start(wq_t[:, kt, :], wq_d[kt * P : (kt + 1) * P, :])
            nv_t = cpool.tile([1, n_gathers], mybir.dt.int32)
            nc.sync.dma_start(nv_t[:], nv_d[:])
            # zero the gather-pool slots once: skipped (-1) gather slots
            # must never expose NaN bit patterns to the 0-weighted matmul
            g_zs = [
                gpool.tile([P, MAX_GATHER_BLOCKS, N_FEAT], gdt, tag="g",
                           name=f"g_z{i}")
                for i in range(3)
            ]
            for g_z in g_zs:
                zt = g_z[:] if gdt == mybir.dt.float16 else g_z[:].bitcast(F32)
                nc.vector.memset(zt, 0.0)

            def body():
                gi = 0  # gather index in program order
                for w in range(n_win):
                    groups = windows[w]
                    bw = sum(nb for _, _, nb in groups)
                    wblk0 = groups[0][1]
                    nd = min(P, rows_per_core - w * P)

                    idx_t = wio.tile([P, max_bw * 8], I16, tag="idx")
                    nc.sync.dma_start(
                        idx_t[:, : bw * 8],
                        gidx_d[:, wblk0 * 8 : (wblk0 + bw) * 8],
                    )
                    offs_t = wio.tile([P, max_bw], F32, tag="offs")
                    nc.sync.dma_start(offs_t[:, :bw], offs_d[:, wblk0 : wblk0 + bw])
                    vals_t = wio.tile([P, max_bw], F32, tag="vals")
                    nc.sync.dma_start(vals_t[:, :bw], vals_d[:, wblk0 : wblk0 + bw])

                    acc_ps = None if skip_compute else psacc.tile(
                        [P, N_FEAT], F32, space="PSUM")
                    bi = 0  # block index within window
                    for ch, blk0, nb in groups:
                        cb = ch * CHUNK
                        x_src = x_d[cb : min(cb + CHUNK, n_nodes), :]
                        done = 0
                        while done < nb:
                            gb = min(MAX_GATHER_BLOCKS, nb - done)
                            g_t = gpool.tile([P, gb, N_FEAT], gdt, tag="g")
                            if not skip_gather:
                                nvv = nc.gpsimd.value_load(
                                    nv_t[0:1, gi : gi + 1]
                                )
                                nc.gpsimd.dma_gather(
                                    out_ap=g_t[:],
                                    in_ap=x_src,
                                    idxs_ap=idx_t[
                                        :, (bi + done) * 8 : (bi + done + gb) * 8
                                    ],
                                    num_idxs=gb * P,
                                    num_idxs_reg=nvv,
                                    elem_size=N_FEAT,
                                )
                            gi += 1
                            for b in range(0 if skip_compute else gb):
                                wb = bi + done + b  # window-block id
                                m_t = mpool.tile([P, P], gdt, tag="m")
                                nc.vector.tensor_scalar(
                                    out=m_t[:],
                                    in0=iota_t[:],
                                    scalar1=offs_t[:, wb : wb + 1],
                                    scalar2=vals_t[:, wb : wb + 1],
                                    op0=mybir.AluOpType.is_equal,
                                    op1=mybir.AluOpType.mult,
                                )
                                nc.tensor.matmul(
                                    out=acc_ps[:],
                                    lhsT=m_t[:],
                                    rhs=g_t[:, b, :],
                                    start=(wb == 0),
                                    stop=(wb == bw - 1),
                                )
                            done += gb
                        bi += nb

                    if skip_compute:
                        continue
                    # evict window: acc -> sbuf -> transpose -> @ wq -> out
                    acc_sb = ev.tile([P, N_FEAT], F32, tag="accsb")
                    nc.vector.tensor_copy(acc_sb[:], acc_ps[:])
                    tr_ps = pstr.tile([P, 4, P], F32, space="PSUM")
                    for fg in range(4):
                        nc.tensor.transpose(
                            out=tr_ps[:, fg, :],
                            in_=acc_sb[:, fg * P : (fg + 1) * P],
                            identity=ident_t[:],
                        )
                    accT = ev.tile([P, 4, P], F32, tag="accT")
                    nc.vector.tensor_copy(accT[:], tr_ps[:])
                    out_ps = psout.tile([P, N_FEAT], F32, space="PSUM")
                    for fg in range(4):
                        nc.tensor.matmul(
                            out=out_ps[:],
                            lhsT=accT[:, fg, :],
                            rhs=wq_t[:, fg, :],
                            start=(fg == 0),
                            stop=(fg == 3),
                        )
                    out_sb = ev.tile([P, N_FEAT], F32, tag="outsb")
                    nc.vector.tensor_copy(out_sb[:], out_ps[:])
                    nc.sync.dma_start(
                        out_d[w * P : w * P + nd, :], out_sb[:nd, :]
                    )

            if reps == 1:
                body()
            else:
                with tc.For_i(0, reps, 1):
                    body()

    nc.compile()
    # safety: every gather-pool tile must land on one of the 3 slots the
    # init memsets zeroed (skipped -1 pad slots must never read boot NaNs)
    init_addrs, g_addrs = set(), set()
    for alloc in nc.m.functions[0].allocations:
        name = getattr(alloc, "name", "") or ""
        mls = getattr(alloc, "memorylocations", None)
        if not mls:
            continue
        if name.startswith("g_z"):
            init_addrs.add(mls[0].addr)
        elif name.startswith("g_t"):
            g_addrs.add(mls[0].addr)
    assert init_addrs and g_addrs and g_addrs <= init_addrs, (
        g_addrs, init_addrs)
    return nc


_IOTA = np.broadcast_to(np.arange(P, dtype=np.float32), (P, P)).copy()
_IDENT = np.eye(P, dtype=np.float32)


def prepare(x, weight, adj_rows, adj_cols, adj_vals, precision="f32r", reps=1,
            skip_compute=False, skip_gather=False):
    """Host prep + program build. Returns (nc, in_maps, meta)."""
    x = np.ascontiguousarray(np.asarray(x, dtype=np.float32))
    if precision == "f16":
        x = x.astype(np.float16)
    n_nodes = x.shape[0]
    wq = _bjorck_host(np.asarray(weight, dtype=np.float32))
    meta, per_core = _prep_host(
        np.asarray(adj_rows), np.asarray(adj_cols),
        np.asarray(adj_vals, dtype=np.float32), n_nodes, N_CORES,
    )
    nc = _build_program(meta, precision=precision, reps=reps,
                        skip_compute=skip_compute, skip_gather=skip_gather)
    in_maps = []
    for c in range(N_CORES):
        in_maps.append(
            {
                "x": x,
                "wq": wq,
                "gidx": per_core[c]["gidx"],
                "nvalid": per_core[c]["nvalid"],
                "offs": per_core[c]["offs"],
                "vals": per_core[c]["vals"],
                "iota": _IOTA,
                "ident": _IDENT,
            }
        )
    return nc, in_maps, meta


def kernel(x, weight, adj_rows, adj_cols, adj_vals):
    nc, in_maps, meta = prepare(x, weight, adj_rows, adj_cols, adj_vals)
    res = run_bass_kernel_spmd(nc, in_maps, list(range(N_CORES)))
    out = np.concatenate([res.results[c]["out"] for c in range(N_CORES)], axis=0)
    return out.astype(np.float32)



# revision 2
# speedup vs baseline: 5.0379x; 5.0379x over previous
"""Trainium2 Bass kernel for nn_BGraphConvolution (GCN message passing).

Computes out = segment_sum(vals * x[cols]) @ bjorck_orthonormalize(weight),
using the associativity out = (A_sp @ x) @ W_ortho:
  - 8-way shard over destination nodes (adj_rows is sorted, so each core
    owns a contiguous edge slab).
  - Per 128-dest window: dma_gather source rows of x from HBM (int16
    indices -> 4 source chunks of 32768 rows), build a one-hot
    stationary matrix M[e, d] = val[e] * (off[e] == d) on the DVE, and
    accumulate PSUM[d, f] += M.T @ G on the PE.
  - Transpose the window accumulator with PE transposes, then multiply
    by the (host-precomputed, fp64 Bjorck) orthonormalized weight.
The Bjorck orthonormalization is a cheap 512x512 fixed-point iteration;
it is computed once on the host in fp64 (exact) and shipped to every
core, per the "replicate the small weight orthonormalization" sharding
strategy.
"""

import os
import sys

for _p in ("/opt/trn_rl_repo", os.path.expanduser("~/.axon_site/_ro/trn_rl_repo")):
    if os.path.isdir(_p) and _p not in sys.path:
        sys.path.insert(0, _p)

import numpy as np

import concourse.mybir as mybir
import concourse.tile as tile
from concourse import bacc
from concourse.bass_utils import run_bass_kernel_spmd

P = 128
N_NODES = 100000
N_FEAT = 512
N_CORES = 8
CHUNK = 32768  # int16 gather index range
BJORCK_BETA = 0.5
BJORCK_ITERS = 10
MAX_GATHER_BLOCKS = 8  # 1024 idxs per dma_gather (SWDGE ring limit)

F32 = mybir.dt.float32
F32R = mybir.dt.float32r
I16 = mybir.dt.int16


def _bjorck_host(weight: np.ndarray) -> np.ndarray:
    """ortho_w in fp64 exactly as the reference defines it."""
    s = float(np.float32(np.sqrt(weight.shape[0] * weight.shape[1])))
    w = weight.astype(np.float64).T / s
    for _ in range(BJORCK_ITERS):
        w = (1.0 + BJORCK_BETA) * w - BJORCK_BETA * (w @ (w.T @ w))
    return np.ascontiguousarray(w.T.astype(np.float32))


def _prep_host(adj_rows, adj_cols, adj_vals, n_nodes, n_cores):
    """Shard edges by destination, group per (dest-window, src-chunk),
    pad each group to 128-blocks with a SHARED (cross-core) structure.

    Returns (meta, per_core) where meta describes the shared program
    structure and per_core holds each core's padded streams.
    """
    rows_per_core = n_nodes // n_cores
    n_win = (rows_per_core + P - 1) // P
    n_chunk = (n_nodes + CHUNK - 1) // CHUNK

    cores = []
    counts = np.zeros((n_cores, n_win * n_chunk), dtype=np.int64)
    for c in range(n_cores):
        lo, hi = np.searchsorted(adj_rows, [c * rows_per_core, (c + 1) * rows_per_core])
        r = adj_rows[lo:hi].astype(np.int64) - c * rows_per_core
        col = adj_cols[lo:hi].astype(np.int64)
        v = adj_vals[lo:hi]
        w = r // P
        ch = col // CHUNK
        key = w * n_chunk + ch
        order = np.argsort(key, kind="stable")
        cores.append((r[order], col[order], v[order], key[order]))
        counts[c] = np.bincount(key, minlength=n_win * n_chunk)

    gmax = counts.max(axis=0)  # shared max per (window, chunk)
    nblk = (gmax + P - 1) // P  # blocks per group (0 = skip group)
    tot_blk = int(nblk.sum())

    # shared structure: per window -> list of (chunk, blk0, nblocks)
    windows = []
    blk0 = 0
    gathers = []  # (key, slice_start_block_global, gb) in program order
    for w in range(n_win):
        groups = []
        for ch in range(n_chunk):
            nb = int(nblk[w * n_chunk + ch])
            if nb:
                groups.append((ch, blk0, nb))
                for i in range(0, nb, MAX_GATHER_BLOCKS):
                    gb = min(MAX_GATHER_BLOCKS, nb - i)
                    gathers.append((w * n_chunk + ch, blk0 + i, gb))
                blk0 += nb
        windows.append(groups)
    assert blk0 == tot_blk

    # per-core padded streams
    group_start = np.zeros(n_win * n_chunk + 1, dtype=np.int64)
    np.cumsum(nblk * P, out=group_start[1:])
    tot = tot_blk * P
    per_core = []
    for c in range(n_cores):
        r, col, v, key = cores[c]
        gidx = np.full(tot, -1, dtype=np.int16)
        offs = np.zeros(tot, dtype=np.float32)
        vals = np.zeros(tot, dtype=np.float32)
        cnt = np.bincount(key, minlength=n_win * n_chunk)
        # position of each edge inside its (padded) group
        pos_in_group = np.arange(len(key)) - np.repeat(
            np.concatenate(([0], np.cumsum(cnt)))[:-1], cnt
        )
        dst = group_start[key] + pos_in_group
        gidx[dst] = (col % CHUNK).astype(np.int16)
        offs[dst] = (r % P).astype(np.float32)
        vals[dst] = v
        # per-gather valid counts (pads are -1 at each group tail and
        # are skipped by the DGE); empty slices get one forced idx 0
        nvalid = np.zeros(len(gathers), dtype=np.int32)
        for gi, (kk, sb, gb) in enumerate(gathers):
            local0 = (sb - (group_start[kk] // P)) * P
            valid = int(min(max(cnt[kk] - local0, 0), gb * P))
            if valid == 0:
                gidx[sb * P] = 0
                valid = 1
            nvalid[gi] = valid
        per_core.append(
            {
                "gidx": np.ascontiguousarray(
                    np.tile(gidx.reshape(-1, 16).T, (8, 1))
                ),  # [128, tot/16]
                "offs": np.ascontiguousarray(offs.reshape(-1, P).T),  # [128, tot_blk]
                "vals": np.ascontiguousarray(vals.reshape(-1, P).T),
                "nvalid": nvalid.reshape(1, -1).copy(),
            }
        )

    meta = {
        "gathers": gathers,
        "n_win": n_win,
        "n_chunk": n_chunk,
        "windows": windows,
        "tot_blk": tot_blk,
        "rows_per_core": rows_per_core,
        "n_nodes": n_nodes,
    }
    return meta, per_core


def _build_program(meta, precision="f32r", reps=1,
                   skip_compute=False, skip_gather=False):
    """Build the per-core Bass program (shared across all cores)."""
    n_win = meta["n_win"]
    windows = meta["windows"]
    tot_blk = meta["tot_blk"]
    rows_per_core = meta["rows_per_core"]
    n_nodes = meta["n_nodes"]

    gdt = {"f32r": F32R, "f32": F32, "f16": mybir.dt.float16}[precision]
    max_bw = max(sum(nb for _, _, nb in groups) for groups in windows)

    nc = bacc.Bacc(
        "TRN2",
        target_bir_lowering=False,
        debug=False,
        num_devices=1,
    )
    x_d = nc.dram_tensor("x", [n_nodes, N_FEAT], gdt, kind="ExternalInput").ap()
    wq_d = nc.dram_tensor("wq", [N_FEAT, N_FEAT], F32, kind="ExternalInput").ap()
    gidx_d = nc.dram_tensor("gidx", [P, tot_blk * 8], I16, kind="ExternalInput").ap()
    offs_d = nc.dram_tensor("offs", [P, tot_blk], F32, kind="ExternalInput").ap()
    vals_d = nc.dram_tensor("vals", [P, tot_blk], F32, kind="ExternalInput").ap()
    iota_d = nc.dram_tensor("iota", [P, P], F32, kind="ExternalInput").ap()
    ident_d = nc.dram_tensor("ident", [P, P], F32, kind="ExternalInput").ap()
    n_gathers = len(meta["gathers"])
    nv_d = nc.dram_tensor(
        "nvalid", [1, n_gathers], mybir.dt.int32, kind="ExternalInput"
    ).ap()
    out_d = nc.dram_tensor(
        "out", [rows_per_core, N_FEAT], F32, kind="ExternalOutput"
    ).ap()

    with tile.TileContext(nc) as tc:
        with (
            tc.tile_pool(name="const", bufs=1) as cpool,
            tc.tile_pool(name="gpool", bufs=3) as gpool,
            tc.tile_pool(name="mpool", bufs=6) as mpool,
            tc.tile_pool(name="wio", bufs=2) as wio,
            tc.tile_pool(name="evict", bufs=2) as ev,
            tc.tile_pool(name="psacc", bufs=2, space="PSUM") as psacc,
            tc.tile_pool(name="pstr", bufs=2, space="PSUM") as pstr,
            tc.tile_pool(name="psout", bufs=2, space="PSUM") as psout,
        ):
            iota_t = cpool.tile([P, P], F32)
            nc.sync.dma_start(iota_t[:], iota_d[:])
            ident_t = cpool.tile([P, P], F32)
            nc.sync.dma_start(ident_t[:], ident_d[:])
            wq_t = cpool.tile([P, 4, N_FEAT], F32)
            for kt in range(4):
                nc.sync.dma_start(wq_t[:, kt, :], wq_d[kt * P : (kt + 1) * P, :])
            nv_t = cpool.tile([1, n_gathers], mybir.dt.int32)
            nc.sync.dma_start(nv_t[:], nv_d[:])
            # zero the gather-pool slots once: skipped (-1) gather slots
            # must never expose NaN bit patterns to the 0-weighted matmul
            g_zs = [
                gpool.tile([P, MAX_GATHER_BLOCKS, N_FEAT], gdt, tag="g",
                           name=f"g_z{i}")
                for i in range(3)
            ]
            for g_z in g_zs:
                zt = g_z[:] if gdt == mybir.dt.float16 else g_z[:].bitcast(F32)
                nc.vector.memset(zt, 0.0)

            def body():
                gi = 0  # gather index in program order
                for w in range(n_win):
                    groups = windows[w]
                    bw = sum(nb for _, _, nb in groups)
                    wblk0 = groups[0][1]
                    nd = min(P, rows_per_core - w * P)

                    idx_t = wio.tile([P, max_bw * 8], I16, tag="idx")
                    nc.sync.dma_start(
                        idx_t[:, : bw * 8],
                        gidx_d[:, wblk0 * 8 : (wblk0 + bw) * 8],
                    )
                    offs_t = wio.tile([P, max_bw], F32, tag="offs")
                    nc.sync.dma_start(offs_t[:, :bw], offs_d[:, wblk0 : wblk0 + bw])
                    vals_t = wio.tile([P, max_bw], F32, tag="vals")
                    nc.sync.dma_start(vals_t[:, :bw], vals_d[:, wblk0 : wblk0 + bw])

                    acc_ps = None if skip_compute else psacc.tile(
                        [P, N_FEAT], F32, space="PSUM")
                    bi = 0  # block index within window
                    for ch, blk0, nb in groups:
                        cb = ch * CHUNK
                        x_src = x_d[cb : min(cb + CHUNK, n_nodes), :]
                        done = 0
                        while done < nb:
                            gb = min(MAX_GATHER_BLOCKS, nb - done)
                            g_t = gpool.tile([P, gb, N_FEAT], gdt, tag="g")
                            if not skip_gather:
                                nvv = nc.gpsimd.value_load(
                                    nv_t[0:1, gi : gi + 1]
                                )
                                nc.gpsimd.dma_gather(
                                    out_ap=g_t[:],
                                    in_ap=x_src,
                                    idxs_ap=idx_t[
                                        :, (bi + done) * 8 : (bi + done + gb) * 8
                                    ],
                                    num_idxs=gb * P,
                                    num_idxs_reg=nvv,
                                    elem_size=N_FEAT,
                                )
                            gi += 1
                            for b in range(0 if skip_compute else gb):
                                wb = bi + done + b  # window-block id
                                m_t = mpool.tile([P, P], gdt, tag="m")
                                nc.vector.tensor_scalar(
                                    out=m_t[:],
                                    in0=iota_t[:],
                                    scalar1=offs_t[:, wb : wb + 1],
                                    scalar2=vals_t[:, wb : wb + 1],
                                    op0=mybir.AluOpType.is_equal,
                                    op1=mybir.AluOpType.mult,
                                )
                                nc.tensor.matmul(
                                    out=acc_ps[:],
                                    lhsT=m_t[:],
                                    rhs=g_t[:, b, :],
                                    start=(wb == 0),
                                    stop=(wb == bw - 1),
                                )
                            done += gb
                        bi += nb

                    if skip_compute:
                        continue
                    # evict window: acc -> sbuf -> transpose -> @ wq -> out
                    acc_sb = ev.tile([P, N_FEAT], F32, tag="accsb")
                    nc.vector.tensor_copy(acc_sb[:], acc_ps[:])
                    tr_ps = pstr.tile([P, 4, P], F32, space="PSUM")
                    for fg in range(4):
                        nc.tensor.transpose(
                            out=tr_ps[:, fg, :],
                            in_=acc_sb[:, fg * P : (fg + 1) * P],
                            identity=ident_t[:],
                        )
                    accT = ev.tile([P, 4, P], F32, tag="accT")
                    nc.vector.tensor_copy(accT[:], tr_ps[:])
                    out_ps = psout.tile([P, N_FEAT], F32, space="PSUM")
                    for fg in range(4):
                        nc.tensor.matmul(
                            out=out_ps[:],
                            lhsT=accT[:, fg, :],
                            rhs=wq_t[:, fg, :],
                            start=(fg == 0),
                            stop=(fg == 3),
                        )
                    out_sb = ev.tile([P, N_FEAT], F32, tag="outsb")
                    nc.vector.tensor_copy(out_sb[:], out_ps[:])
                    nc.sync.dma_start(
                        out_d[w * P : w * P + nd, :], out_sb[:nd, :]
                    )

            if reps == 1:
                body()
            else:
                with tc.For_i(0, reps, 1):
                    body()

    nc.compile()
    # safety: every gather-pool tile must land on one of the 3 slots the
    # init memsets zeroed (skipped -1 pad slots must never read boot NaNs)
    init_addrs, g_addrs = set(), set()
    for alloc in nc.m.functions[0].allocations:
        name = getattr(alloc, "name", "") or ""
        mls = getattr(alloc, "memorylocations", None)
        if not mls:
            continue
        if name.startswith("g_z"):
            init_addrs.add(mls[0].addr)
        elif name.startswith("g_t"):
            g_addrs.add(mls[0].addr)
    assert init_addrs and g_addrs and g_addrs <= init_addrs, (
        g_addrs, init_addrs)
    return nc


_IOTA = np.broadcast_to(np.arange(P, dtype=np.float32), (P, P)).copy()
_IDENT = np.eye(P, dtype=np.float32)


def prepare(x, weight, adj_rows, adj_cols, adj_vals, precision="f16", reps=1,
            skip_compute=False, skip_gather=False):
    """Host prep + program build. Returns (nc, in_maps, meta)."""
    x = np.ascontiguousarray(np.asarray(x, dtype=np.float32))
    if precision == "f16":
        x = x.astype(np.float16)
    n_nodes = x.shape[0]
    wq = _bjorck_host(np.asarray(weight, dtype=np.float32))
    meta, per_core = _prep_host(
        np.asarray(adj_rows), np.asarray(adj_cols),
        np.asarray(adj_vals, dtype=np.float32), n_nodes, N_CORES,
    )
    nc = _build_program(meta, precision=precision, reps=reps,
                        skip_compute=skip_compute, skip_gather=skip_gather)
    in_maps = []
    for c in range(N_CORES):
        in_maps.append(
            {
                "x": x,
                "wq": wq,
                "gidx": per_core[c]["gidx"],
                "nvalid": per_core[c]["nvalid"],
                "offs": per_core[c]["offs"],
                "vals": per_core[c]["vals"],
                "iota": _IOTA,
                "ident": _IDENT,
            }
        )
    return nc, in_maps, meta


def kernel(x, weight, adj_rows, adj_cols, adj_vals):
    nc, in_maps, meta = prepare(x, weight, adj_rows, adj_cols, adj_vals)
    res = run_bass_kernel_spmd(nc, in_maps, list(range(N_CORES)))
    out = np.concatenate([res.results[c]["out"] for c in range(N_CORES)], axis=0)
    return out.astype(np.float32)

